# revision 12
# baseline (speedup 1.0000x reference)
"""Trainium2 Bass kernel for nn_DWT_Layer: 3-level 2D db4 DWT (symmetric mode).

Input  x: (16, 3, 1024, 1024) fp32.
Output:   (16, 3, 64, 128, 128) fp32 — the flattened/truncated wavelet pyramid
          [cA3, cH3, cV3, cD3, cH2, cV2, cD2, cH1, cV1, cD1(truncated)].

Sharding: pure data parallel — the 48 (batch*channel) images are split 6 per
NeuronCore across 8 cores; no communication.

Per-core dataflow, per image, per level (N -> N' = floor((N+5)/2)+1):
  1. width pass along the free axis: 8-tap stride-2 MAC chains on the
     vector (DVE) + gpsimd engines over a symmetric-extension buffer
     (ext built by DMA/copies writing the interior + 4 mirror copies).
  2. height pass as banded fp32 matmuls on the tensor engine: stacked
     [lo; hi] folded DWT matrix (symmetric fold absorbed into weights),
     contraction over partitions; only nonzero 128x128 blocks are run.
  3. scalar (ACT) engine copies PSUM -> SBUF, splitting quadrants; the
     aa quadrant lands in the next level's ext buffer, detail quadrants
     land in per-slot staging tiles that stream to DRAM.
Everything comes out h-major so output DMAs are contiguous-row writes.
"""
import numpy as np

# ----------------------------------------------------------------- constants
DEC_LO = np.array([-0.010597401784997278, 0.032883011666982945,
                   0.030841381835986965, -0.18703481171888114,
                   -0.027983769416983849, 0.63088076792959036,
                   0.71484657055254153, 0.23037781330885523], dtype=np.float64)
L = 8
DEC_HI = np.array([(-1.0) ** (k + 1) * DEC_LO[L - 1 - k] for k in range(L)],
                  dtype=np.float64)
FREV_LO = [float(v) for v in DEC_LO[::-1].astype(np.float32)]
FREV_HI = [float(v) for v in DEC_HI[::-1].astype(np.float32)]

B, C, H, W = 16, 3, 1024, 1024
N_CORES = 8
IMGS_PER_CORE = 6
IMG_ELEMS = H * W

LEVELS = [  # (N, N', n_slots_in, n_out_tiles)
    (1024, 515, 8, 9),
    (515, 261, 5, 5),
    (261, 134, 3, 3),
]

# output section offsets (elements within one image's 1048576-long output)
SECT = {}
_cur = 0
for _name, _n in [("cA3", 134), ("cH3", 134), ("cV3", 134), ("cD3", 134),
                  ("cH2", 261), ("cV2", 261), ("cD2", 261),
                  ("cH1", 515), ("cV1", 515), ("cD1", 515)]:
    SECT[_name] = (_cur, _n)
    _cur += _n * _n
# cD1 truncation: keep first 469 full rows + 404 elems of row 469
CD1_FULL_ROWS = 469
CD1_PART_COLS = 404
assert SECT["cD1"][0] + CD1_FULL_ROWS * 515 + CD1_PART_COLS == IMG_ELEMS


def nprime(N):
    return (N + 5) // 2 + 1


def ext_index(j, N):
    if j < 6:
        return 5 - j
    if j < N + 6:
        return j - 6
    return 2 * N + 5 - j


def dwt_matrix(N, filt):
    Np = nprime(N)
    M = np.zeros((Np, N), dtype=np.float64)
    filtrev = filt[::-1]
    for i in range(Np):
        for t in range(L):
            M[i, ext_index(2 * i + t, N)] += filtrev[t]
    return M


def hi_off(Np):
    """row offset of the hi section, padded to a multiple of 32 so that
    engine ops on the hi quadrant start at partition 32/64/0 (BIR verifier:
    SBUF engine APs must start at partition 0/32/64/96)."""
    return ((Np + 31) // 32) * 32


def stacked_matrix(N):
    Np = nprime(N)
    off = hi_off(Np)
    M2 = np.zeros((off + Np, N), dtype=np.float64)
    M2[0:Np] = dwt_matrix(N, DEC_LO)
    M2[off:] = dwt_matrix(N, DEC_HI)
    return M2.astype(np.float32)


def band_blocks(N):
    """[(t, q, kq, mt, band_pos)]: nonzero blocks of M2^T; band_pos tags
    first/last per (t) for start/stop flags."""
    M2 = stacked_matrix(N)
    R = M2.shape[0]
    kt = (N + 127) // 128
    ot = (R + 127) // 128
    per_t = []
    for t in range(ot):
        qs = []
        for q in range(kt):
            blk = M2[t * 128:(t + 1) * 128, q * 128:(q + 1) * 128]
            if np.any(blk != 0):
                qs.append(q)
        per_t.append(qs)
    return per_t, kt, ot, R


def const_weights(N):
    """packed lhsT blocks [128, nblocks, 128] + index map {(t,q): b}."""
    M2 = stacked_matrix(N)
    per_t, kt, ot, R = band_blocks(N)
    blocks = [(t, q) for t in range(ot) for q in per_t[t]]
    arr = np.zeros((128, len(blocks), 128), dtype=np.float32)
    idx = {}
    for b, (t, q) in enumerate(blocks):
        blk = M2[t * 128:(t + 1) * 128, q * 128:(q + 1) * 128]  # [mt, kq]
        arr[:blk.shape[1], b, :blk.shape[0]] = blk.T
        idx[(t, q)] = b
    return arr, idx, per_t


WC = {N: const_weights(N) for N, _, _, _ in LEVELS}

# DVE vs GPSIMD split of the interior MAC columns
DVE_FRAC = 0.55

_BUILT = None  # cached (nc, meta)


def _free_chunks(Np):
    """quadrant-aligned free chunks of <=512: [(c0, c1), ...] covering
    [0:2Np). Each chunk lies in one quadrant and fits one PSUM bank."""
    out = []
    for base in (0, Np):
        c = 0
        while c < Np:
            e = min(c + 512, Np)
            out.append((base + c, base + e))
            c = e
    return out


def _emit_mirror_ops(nc, ext, S, N):
    """Fill ext cols [0:6) and [N+6:N+13) from the interior [6:N+6)."""
    # left: ext[j] = x[5-j] = ext[6 + 5 - j] -> reversed slice of cols [6:12)
    nc.vector.tensor_copy(out=ext[:, 0:S, 0:6], in_=ext[:, 0:S, 11:5:-1])
    # right: ext[N+6+k] = x[N-1-k] at ext col N+5-k -> reversed (N+5 .. N-1)
    nc.vector.tensor_copy(out=ext[:, 0:S, N + 6:N + 13],
                          in_=ext[:, 0:S, N + 5:N - 2:-1])


def _emit_mac_pass(nc, ext, wb, S, N, Np):
    """width pass: wb[:, s, c + base] = sum_t frev[t] * ext[:, s, 2c+t].

    All on the vector engine: walrus rejects TensorScalarPtr on Pool
    (gpsimd), and ACT has no tensor-accumulate op."""
    import concourse.mybir as mybir
    ranges = [(nc.vector, 0, Np)]
    for fi, frev in enumerate((FREV_LO, FREV_HI)):
        base = fi * Np
        for eng, c0, c1 in ranges:
            if c0 >= c1:
                continue
            n = c1 - c0
            for t in range(L):
                src = ext[:, 0:S, 2 * c0 + t: 2 * c0 + t + 2 * (n - 1) + 1: 2]
                dst = wb[:, 0:S, base + c0: base + c1]
                if t == 0:
                    eng.tensor_scalar_mul(dst, src, frev[t])
                else:
                    eng.scalar_tensor_tensor(
                        out=dst, in0=src, scalar=frev[t], in1=dst,
                        op0=mybir.AluOpType.mult, op1=mybir.AluOpType.add)


def build_bass(n_images=IMGS_PER_CORE):
    import concourse.mybir as mybir
    import concourse.tile as tile
    from concourse import bacc
    from contextlib import ExitStack

    nc = bacc.Bacc("TRN2", target_bir_lowering=False, debug=False)

    xin = nc.dram_tensor("xin", (n_images, H, W), mybir.dt.float32,
                         kind="ExternalInput").ap()
    out = nc.dram_tensor("out", (n_images, IMG_ELEMS), mybir.dt.float32,
                         kind="ExternalOutput").ap()
    wdram = {}
    for N, _, _, _ in LEVELS:
        arr, _, _ = WC[N]
        wdram[N] = nc.dram_tensor(f"w{N}", arr.shape, mybir.dt.float32,
                                  kind="ExternalInput").ap()

    with tile.TileContext(nc) as tc, ExitStack() as ctx:
        cpool = ctx.enter_context(tc.tile_pool(name="consts", bufs=1))
        extp = ctx.enter_context(tc.tile_pool(name="ext", bufs=1))
        wbp = ctx.enter_context(tc.tile_pool(name="wb", bufs=1))
        psp = ctx.enter_context(tc.tile_pool(name="ps", bufs=1, space="PSUM"))
        detp = ctx.enter_context(tc.tile_pool(name="det", bufs=1))

        wsb = {}
        for N, _, _, _ in LEVELS:
            arr, _, _ = WC[N]
            wsb[N] = cpool.tile(list(arr.shape), mybir.dt.float32,
                                name=f"wsb{N}")
            nc.sync.dma_start(out=wsb[N][:], in_=wdram[N])

        for img in range(n_images):
            _emit_image(nc, tc, extp, wbp, psp, detp, wsb, xin, out, img)

    nc.compile()
    return nc


def _emit_image(nc, tc, extp, wbp, psp, detp, wsb, xin, out, img):
    import concourse.mybir as mybir

    N1, P1 = 1024, 515
    # ---------------- L1: ext halves + MACs ----------------
    halves = []
    for h in range(2):
        ext = extp.tile([128, 4, N1 + 13], mybir.dt.float32, tag="ext1",
                        bufs=2, name=f"ext1_{img}_{h}")
        src = xin[img, 512 * h:512 * (h + 1), :].rearrange(
            "(s p) w -> p s w", p=128)
        nc.sync.dma_start(out=ext[:, 0:4, 6:N1 + 6], in_=src)
        _emit_mirror_ops(nc, ext, 4, N1)
        wb = wbp.tile([128, 4, 2 * P1], mybir.dt.float32, tag="wb1",
                      bufs=2, name=f"wb1_{img}_{h}")
        _emit_mac_pass(nc, ext, wb, 4, N1, P1)
        halves.append(wb)

    def rhs1(q, c0, c1):
        return halves[q // 4][:, q % 4, c0:c1]

    # next-level ext buffers; memset the partial last slot so the unwritten
    # partitions (beyond the valid rows) are finite zeros
    ext2 = extp.tile([128, 5, 515 + 13], mybir.dt.float32, tag="ext2",
                     bufs=2, name=f"ext2_{img}")
    nc.vector.memset(ext2[:, 4, :], 0.0)
    ext3 = extp.tile([128, 3, 261 + 13], mybir.dt.float32, tag="ext3",
                     bufs=2, name=f"ext3_{img}")
    nc.vector.memset(ext3[:, 2, :], 0.0)

    _emit_level_mm(nc, psp, detp, wsb, out, img, N=1024, rhs=rhs1,
                   next_ext=ext2, det_names=("cH1", "cV1", "cD1"))
    _emit_mirror_ops(nc, ext2, 5, 515)

    wb2 = wbp.tile([128, 5, 2 * 261], mybir.dt.float32, tag="wb2",
                   bufs=2, name=f"wb2_{img}")
    _emit_mac_pass(nc, ext2, wb2, 5, 515, 261)

    def rhs2(q, c0, c1):
        return wb2[:, q, c0:c1]

    _emit_level_mm(nc, psp, detp, wsb, out, img, N=515, rhs=rhs2,
                   next_ext=ext3, det_names=("cH2", "cV2", "cD2"))
    _emit_mirror_ops(nc, ext3, 3, 261)

    wb3 = wbp.tile([128, 3, 2 * 134], mybir.dt.float32, tag="wb3",
                   bufs=2, name=f"wb3_{img}")
    _emit_mac_pass(nc, ext3, wb3, 3, 261, 134)

    def rhs3(q, c0, c1):
        return wb3[:, q, c0:c1]

    _emit_level_mm(nc, psp, detp, wsb, out, img, N=261, rhs=rhs3,
                   next_ext=None, det_names=("cH3", "cV3", "cD3"))


def _emit_level_mm(nc, psp, detp, wsb, out, img, N, rhs, next_ext, det_names):
    """height-pass matmuls + psum->sbuf quadrant copies + detail DMAs."""
    import concourse.mybir as mybir

    Np = nprime(N)
    arr, idx, per_t = WC[N]
    OFF = hi_off(Np)
    R = OFF + Np
    ot = (R + 127) // 128
    kN = N  # contraction length
    chunks = _free_chunks(Np)

    for t in range(ot):
        mt = min(128, R - t * 128)
        qs = per_t[t]
        ps_tiles = []
        for ci, (c0, c1) in enumerate(chunks):
            w = c1 - c0
            tag = "psA" if w > 256 else "psB"
            ps = psp.tile([128, w], mybir.dt.float32, tag=tag, bufs=4,
                          name=f"ps_{img}_{N}_{t}_{ci}")
            ps_tiles.append(ps)
            for ki, q in enumerate(qs):
                kq = min(128, kN - q * 128)
                r = rhs(q, c0, c1)
                if kq < 128:
                    r = r[0:kq]
                nc.tensor.matmul(
                    ps[0:mt, 0:w],
                    wsb[N][0:kq, idx[(t, q)], 0:mt],
                    r,
                    start=(ki == 0), stop=(ki == len(qs) - 1))

        # quadrant qd -> list of (ps_tile, dst_col0, width)
        quad_srcs = {0: [], 1: []}
        for ci, (c0, c1) in enumerate(chunks):
            qd = 0 if c0 < Np else 1
            quad_srcs[qd].append((ps_tiles[ci], c0 - qd * Np, c1 - c0))

        # lo rows: global [0:Np); hi rows: global [OFF:OFF+Np)
        lo_end = min(128, Np - t * 128) if t * 128 < Np else 0
        hp0 = max(0, OFF - t * 128)
        hp1 = max(0, min(128, OFF + Np - t * 128))
        # split hi ranges at legal partition starts (0/32/64)
        hi_ranges = []
        if hp0 < hp1:
            if hp0 == 0:
                hi_ranges = [(0, hp1)]
            else:
                assert hp0 == 32, hp0
                hi_ranges = [(32, min(64, hp1))]
                if hp1 > 64:
                    hi_ranges.append((64, hp1))

        if lo_end > 0:
            # quadrant 0 = aa -> next level ext (or cA3 staging tile)
            if next_ext is not None:
                for ps, d0, w in quad_srcs[0]:
                    nc.scalar.copy(out=next_ext[0:lo_end, t, 6 + d0:6 + d0 + w],
                                   in_=ps[0:lo_end, 0:w])
            else:
                _emit_det_copy_dma(nc, detp, out, img, "cA3", Np,
                                   quad_srcs[0], t, [(0, lo_end)], 0)
            # quadrant 1 = ad = cV
            _emit_det_copy_dma(nc, detp, out, img, det_names[1], Np,
                               quad_srcs[1], t, [(0, lo_end)], 0)
        if hi_ranges:
            # hi rows: da = cH (quadrant 0), dd = cD (quadrant 1)
            _emit_det_copy_dma(nc, detp, out, img, det_names[0], Np,
                               quad_srcs[0], t, hi_ranges, OFF)
            _emit_det_copy_dma(nc, detp, out, img, det_names[2], Np,
                               quad_srcs[1], t, hi_ranges, OFF)


def _emit_det_copy_dma(nc, detp, out, img, sec_name, Np, srcs, t, pranges,
                       row_off):
    """Copy psum chunks into a staging tile, then DMA rows to DRAM.

    h (row index within the detail) = 128*t + p - row_off for partition p.
    pranges: list of legal-start partition ranges covering this tile's rows."""
    import concourse.mybir as mybir
    sec_base, Wd = SECT[sec_name]
    assert Wd == Np
    p0, p1 = pranges[0][0], pranges[-1][1]
    h0 = 128 * t + p0 - row_off
    h1 = h0 + (p1 - p0)
    assert 0 <= h0 and h1 <= Np, (sec_name, t, pranges, h0, h1)

    is_cd1 = sec_name == "cD1"
    if is_cd1 and h0 >= CD1_FULL_ROWS + 1:
        return  # fully truncated
    dt = detp.tile([128, Np], mybir.dt.float32, tag=f"det{Np}", bufs=6,
                   name=f"det_{sec_name}_{img}_{t}_{p0}")
    for ps, d0, w in srcs:
        for (a, b) in pranges:
            nc.scalar.copy(out=dt[a:b, d0:d0 + w], in_=ps[a:b, 0:w])

    full_h1 = h1
    if is_cd1 and h1 > CD1_FULL_ROWS:
        full_h1 = CD1_FULL_ROWS
    if full_h1 > h0:
        npart = full_h1 - h0
        dst = out[img, sec_base + h0 * Wd: sec_base + full_h1 * Wd].rearrange(
            "(h w) -> h w", w=Wd)
        nc.sync.dma_start(out=dst, in_=dt[p0:p0 + npart, :])
    if is_cd1 and h0 <= CD1_FULL_ROWS < h1:
        pp = p0 + (CD1_FULL_ROWS - h0)
        dst = out[img, sec_base + CD1_FULL_ROWS * Wd:
                  sec_base + CD1_FULL_ROWS * Wd + CD1_PART_COLS]
        nc.sync.dma_start(out=dst.rearrange("(h w) -> h w", w=CD1_PART_COLS),
                          in_=dt[pp:pp + 1, 0:CD1_PART_COLS])


# ----------------------------------------------------------------- runner
def _get_built():
    global _BUILT
    if _BUILT is None:
        _BUILT = build_bass()
    return _BUILT


def kernel(x: np.ndarray) -> np.ndarray:
    from concourse import bass_utils

    x = np.ascontiguousarray(np.asarray(x), dtype=np.float32)
    assert x.shape == (B, C, H, W), x.shape
    nc = _get_built()

    imgs = x.reshape(B * C, H, W)
    in_maps = []
    for c in range(N_CORES):
        m = {"xin": imgs[c * IMGS_PER_CORE:(c + 1) * IMGS_PER_CORE]}
        for N, _, _, _ in LEVELS:
            m[f"w{N}"] = WC[N][0]
        in_maps.append(m)

    res = bass_utils.run_bass_kernel_spmd(nc, in_maps,
                                          core_ids=list(range(N_CORES)))
    outs = [res.results[c]["out"] for c in range(N_CORES)]
    flat = np.concatenate(outs, axis=0)  # [48, 1048576]
    return flat.reshape(B, C, 64, 128, 128)


# revision 22
# speedup vs baseline: 74.9176x; 74.9176x over previous
"""Trainium2 Bass kernel for nn_DWT_Layer: 3-level 2D db4 DWT (symmetric mode).

Input  x: (16, 3, 1024, 1024) fp32.
Output:   (16, 3, 64, 128, 128) fp32 — the flattened/truncated wavelet pyramid
          [cA3, cH3, cV3, cD3, cH2, cV2, cD2, cH1, cV1, cD1(truncated)].

Sharding: pure data parallel — the 48 (batch*channel) images are split 6 per
NeuronCore across 8 cores; no communication.

Per-core dataflow, per image, per level (N -> N' = floor((N+5)/2)+1):
  1. width pass along the free axis: 8-tap stride-2 MAC chains on the
     vector (DVE) + gpsimd engines over a symmetric-extension buffer
     (ext built by DMA/copies writing the interior + 4 mirror copies).
  2. height pass as banded fp32 matmuls on the tensor engine: stacked
     [lo; hi] folded DWT matrix (symmetric fold absorbed into weights),
     contraction over partitions; only nonzero 128x128 blocks are run.
  3. scalar (ACT) engine copies PSUM -> SBUF, splitting quadrants; the
     aa quadrant lands in the next level's ext buffer, detail quadrants
     land in per-slot staging tiles that stream to DRAM.
Everything comes out h-major so output DMAs are contiguous-row writes.
"""
import numpy as np

# ----------------------------------------------------------------- constants
DEC_LO = np.array([-0.010597401784997278, 0.032883011666982945,
                   0.030841381835986965, -0.18703481171888114,
                   -0.027983769416983849, 0.63088076792959036,
                   0.71484657055254153, 0.23037781330885523], dtype=np.float64)
L = 8
DEC_HI = np.array([(-1.0) ** (k + 1) * DEC_LO[L - 1 - k] for k in range(L)],
                  dtype=np.float64)
FREV_LO = [float(v) for v in DEC_LO[::-1].astype(np.float32)]
FREV_HI = [float(v) for v in DEC_HI[::-1].astype(np.float32)]
TAPS_ARR = np.tile(np.array(FREV_LO + FREV_HI, dtype=np.float32)[None, :],
                   (128, 1))

B, C, H, W = 16, 3, 1024, 1024
N_CORES = 8
IMGS_PER_CORE = 6
IMG_ELEMS = H * W

LEVELS = [  # (N, N', n_slots_in, n_out_tiles)
    (1024, 515, 8, 9),
    (515, 261, 5, 5),
    (261, 134, 3, 3),
]

# output section offsets (elements within one image's 1048576-long output)
SECT = {}
_cur = 0
for _name, _n in [("cA3", 134), ("cH3", 134), ("cV3", 134), ("cD3", 134),
                  ("cH2", 261), ("cV2", 261), ("cD2", 261),
                  ("cH1", 515), ("cV1", 515), ("cD1", 515)]:
    SECT[_name] = (_cur, _n)
    _cur += _n * _n
# cD1 truncation: keep first 469 full rows + 404 elems of row 469
CD1_FULL_ROWS = 469
CD1_PART_COLS = 404
assert SECT["cD1"][0] + CD1_FULL_ROWS * 515 + CD1_PART_COLS == IMG_ELEMS


def nprime(N):
    return (N + 5) // 2 + 1


def ext_index(j, N):
    if j < 6:
        return 5 - j
    if j < N + 6:
        return j - 6
    return 2 * N + 5 - j


def dwt_matrix(N, filt):
    Np = nprime(N)
    M = np.zeros((Np, N), dtype=np.float64)
    filtrev = filt[::-1]
    for i in range(Np):
        for t in range(L):
            M[i, ext_index(2 * i + t, N)] += filtrev[t]
    return M


def hi_off(Np):
    """row offset of the hi section, padded to a multiple of 32 so that
    engine ops on the hi quadrant start at partition 32/64/0 (BIR verifier:
    SBUF engine APs must start at partition 0/32/64/96)."""
    return ((Np + 31) // 32) * 32


def stacked_matrix(N):
    Np = nprime(N)
    off = hi_off(Np)
    M2 = np.zeros((off + Np, N), dtype=np.float64)
    M2[0:Np] = dwt_matrix(N, DEC_LO)
    M2[off:] = dwt_matrix(N, DEC_HI)
    return M2.astype(np.float32)


def band_blocks(N):
    """[(t, q, kq, mt, band_pos)]: nonzero blocks of M2^T; band_pos tags
    first/last per (t) for start/stop flags."""
    M2 = stacked_matrix(N)
    R = M2.shape[0]
    kt = (N + 127) // 128
    ot = (R + 127) // 128
    per_t = []
    for t in range(ot):
        qs = []
        for q in range(kt):
            blk = M2[t * 128:(t + 1) * 128, q * 128:(q + 1) * 128]
            if np.any(blk != 0):
                qs.append(q)
        per_t.append(qs)
    return per_t, kt, ot, R


def const_weights(N):
    """packed lhsT blocks [128, nblocks, 128] + index map {(t,q): b}."""
    M2 = stacked_matrix(N)
    per_t, kt, ot, R = band_blocks(N)
    blocks = [(t, q) for t in range(ot) for q in per_t[t]]
    arr = np.zeros((128, len(blocks), 128), dtype=np.float32)
    idx = {}
    for b, (t, q) in enumerate(blocks):
        blk = M2[t * 128:(t + 1) * 128, q * 128:(q + 1) * 128]  # [mt, kq]
        arr[:blk.shape[1], b, :blk.shape[0]] = blk.T
        idx[(t, q)] = b
    return arr, idx, per_t


WC = {N: const_weights(N) for N, _, _, _ in LEVELS}

# ---- MAC pass tuning knobs ----
GP_FRAC = 0.0     # fraction of width-pass columns offloaded to gpsimd (Pool)
MAC_SPLIT = 1     # independent DVE chains per filter (hides RAW ack latency)
TAP0_ACT = True   # first tap (overwrite mul) on the scalar engine
MIRROR_GP = True  # mirror/memset ops on gpsimd instead of DVE
TAP_MAJOR = True  # emit MAC ops tap-major (interleave chains) vs unit-major
EXT1_BUFS = 3
WB1_BUFS = 3
EXT2_BUFS = 1
WB2_BUFS = 1
DET_BUFS = 6

_BUILT = None  # cached (nc, meta)


def _free_chunks(Np):
    """quadrant-aligned free chunks of <=512: [(c0, c1), ...] covering
    [0:2Np). Each chunk lies in one quadrant and fits one PSUM bank."""
    out = []
    for base in (0, Np):
        c = 0
        while c < Np:
            e = min(c + 512, Np)
            out.append((base + c, base + e))
            c = e
    return out


def _emit_mirror_ops(nc, ext, S, N):
    """Fill ext cols [0:6) and [N+6:N+13) from the interior [6:N+6)."""
    eng = nc.gpsimd if MIRROR_GP else nc.vector
    # left: ext[j] = x[5-j] = ext[6 + 5 - j] -> reversed slice of cols [6:12)
    eng.tensor_copy(out=ext[:, 0:S, 0:6], in_=ext[:, 0:S, 11:5:-1])
    # right: ext[N+6+k] = x[N-1-k] at ext col N+5-k -> reversed (N+5 .. N-1)
    eng.tensor_copy(out=ext[:, 0:S, N + 6:N + 13],
                    in_=ext[:, 0:S, N + 5:N - 2:-1])


def _emit_mac_pass(nc, ext, wb, S, N, Np, taps_sb, tmp_pool, lvl):
    """width pass: wb[:, s, c + base] = sum_t frev[t] * ext[:, s, 2c+t].

    DVE runs fused multiply-accumulate (scalar_tensor_tensor) chains;
    a GP_FRAC column share goes to gpsimd as mult+add pairs (walrus
    rejects TensorScalarPtr on Pool). Ops are emitted tap-major so
    independent chains interleave and hide the RAW pipeline latency."""
    import concourse.mybir as mybir
    gp_n = int(Np * GP_FRAC)
    dve_n = Np - gp_n
    units = []  # (kind, fi, c0, c1, tmp)
    nsub = max(1, MAC_SPLIT)
    bounds = [round(dve_n * i / nsub) for i in range(nsub + 1)]
    for fi in (0, 1):
        for si in range(nsub):
            if bounds[si] < bounds[si + 1]:
                units.append(("v", fi, bounds[si], bounds[si + 1], None))
        if gp_n > 0:
            tmp = tmp_pool.tile([128, S, gp_n], mybir.dt.float32,
                                tag=f"gtmp{lvl}", bufs=3,
                                name=f"gtmp{lvl}_{fi}")
            units.append(("g", fi, dve_n, Np, tmp))

    order = ([(t, u) for t in range(L) for u in units] if TAP_MAJOR
             else [(t, u) for u in units for t in range(L)])
    for t, u in order:
        if True:
            kind, fi, c0, c1, tmp = u
            frev = FREV_LO if fi == 0 else FREV_HI
            n = c1 - c0
            base = fi * Np
            src = ext[:, 0:S, 2 * c0 + t: 2 * c0 + t + 2 * (n - 1) + 1: 2]
            dst = wb[:, 0:S, base + c0: base + c1]
            if kind == "v":
                if t == 0:
                    if TAP0_ACT:
                        nc.scalar.mul(dst, src, frev[t])
                    else:
                        nc.vector.tensor_scalar_mul(dst, src, frev[t])
                else:
                    nc.vector.scalar_tensor_tensor(
                        out=dst, in0=src, scalar=frev[t], in1=dst,
                        op0=mybir.AluOpType.mult, op1=mybir.AluOpType.add)
            else:
                btap = taps_sb[:, fi * 8 + t:fi * 8 + t + 1].to_broadcast(
                    (128, S, n))
                if t == 0:
                    nc.gpsimd.tensor_tensor(out=dst, in0=src, in1=btap,
                                            op=mybir.AluOpType.mult)
                else:
                    nc.gpsimd.tensor_tensor(out=tmp[:, 0:S, 0:n], in0=src,
                                            in1=btap, op=mybir.AluOpType.mult)
                    nc.gpsimd.tensor_tensor(out=dst, in0=dst,
                                            in1=tmp[:, 0:S, 0:n],
                                            op=mybir.AluOpType.add)


def build_bass(n_images=IMGS_PER_CORE, repeats=1):
    import concourse.mybir as mybir
    import concourse.tile as tile
    from concourse import bacc
    from contextlib import ExitStack

    nc = bacc.Bacc("TRN2", target_bir_lowering=False, debug=False)

    xin = nc.dram_tensor("xin", (n_images, H, W), mybir.dt.float32,
                         kind="ExternalInput").ap()
    out = nc.dram_tensor("out", (n_images, IMG_ELEMS), mybir.dt.float32,
                         kind="ExternalOutput").ap()
    wdram = {}
    for N, _, _, _ in LEVELS:
        arr, _, _ = WC[N]
        wdram[N] = nc.dram_tensor(f"w{N}", arr.shape, mybir.dt.float32,
                                  kind="ExternalInput").ap()
    taps_dram = nc.dram_tensor("taps", (128, 16), mybir.dt.float32,
                               kind="ExternalInput").ap()

    with tile.TileContext(nc) as tc, ExitStack() as ctx:
        cpool = ctx.enter_context(tc.tile_pool(name="consts", bufs=1))
        extp = ctx.enter_context(tc.tile_pool(name="ext", bufs=1))
        wbp = ctx.enter_context(tc.tile_pool(name="wb", bufs=1))
        psp = ctx.enter_context(tc.tile_pool(name="ps", bufs=1, space="PSUM"))
        detp = ctx.enter_context(tc.tile_pool(name="det", bufs=1))

        wsb = {}
        for N, _, _, _ in LEVELS:
            arr, _, _ = WC[N]
            wsb[N] = cpool.tile(list(arr.shape), mybir.dt.float32,
                                name=f"wsb{N}")
            nc.sync.dma_start(out=wsb[N][:], in_=wdram[N])
        taps_sb = cpool.tile([128, 16], mybir.dt.float32, name="taps_sb")
        nc.sync.dma_start(out=taps_sb[:], in_=taps_dram)

        for _rep in range(repeats):
            for img in range(n_images):
                _emit_image(nc, tc, extp, wbp, psp, detp, wsb, taps_sb,
                            xin, out, img)

    nc.compile()
    return nc


def _emit_image(nc, tc, extp, wbp, psp, detp, wsb, taps_sb, xin, out, img):
    import concourse.mybir as mybir

    N1, P1 = 1024, 515
    # ---------------- L1: ext halves + MACs ----------------
    halves = []
    for h in range(2):
        ext = extp.tile([128, 4, N1 + 13], mybir.dt.float32, tag="ext1",
                        bufs=EXT1_BUFS, name=f"ext1_{img}_{h}")
        src = xin[img, 512 * h:512 * (h + 1), :].rearrange(
            "(s p) w -> p s w", p=128)
        nc.sync.dma_start(out=ext[:, 0:4, 6:N1 + 6], in_=src)
        _emit_mirror_ops(nc, ext, 4, N1)
        wb = wbp.tile([128, 4, 2 * P1], mybir.dt.float32, tag="wb1",
                      bufs=WB1_BUFS, name=f"wb1_{img}_{h}")
        _emit_mac_pass(nc, ext, wb, 4, N1, P1, taps_sb, wbp, 1)
        halves.append(wb)

    def rhs1(q, c0, c1):
        return halves[q // 4][:, q % 4, c0:c1]

    # next-level ext buffers; memset the partial last slot so the unwritten
    # partitions (beyond the valid rows) are finite zeros
    ext2 = extp.tile([128, 5, 515 + 13], mybir.dt.float32, tag="ext2",
                     bufs=EXT2_BUFS, name=f"ext2_{img}")
    (nc.gpsimd if MIRROR_GP else nc.vector).memset(ext2[:, 4, :], 0.0)
    ext3 = extp.tile([128, 3, 261 + 13], mybir.dt.float32, tag="ext3",
                     bufs=EXT2_BUFS, name=f"ext3_{img}")
    (nc.gpsimd if MIRROR_GP else nc.vector).memset(ext3[:, 2, :], 0.0)

    _emit_level_mm(nc, psp, detp, wsb, out, img, N=1024, rhs=rhs1,
                   next_ext=ext2, det_names=("cH1", "cV1", "cD1"))
    _emit_mirror_ops(nc, ext2, 5, 515)

    wb2 = wbp.tile([128, 5, 2 * 261], mybir.dt.float32, tag="wb2",
                   bufs=WB2_BUFS, name=f"wb2_{img}")
    _emit_mac_pass(nc, ext2, wb2, 5, 515, 261, taps_sb, wbp, 2)

    def rhs2(q, c0, c1):
        return wb2[:, q, c0:c1]

    _emit_level_mm(nc, psp, detp, wsb, out, img, N=515, rhs=rhs2,
                   next_ext=ext3, det_names=("cH2", "cV2", "cD2"))
    _emit_mirror_ops(nc, ext3, 3, 261)

    wb3 = wbp.tile([128, 3, 2 * 134], mybir.dt.float32, tag="wb3",
                   bufs=WB2_BUFS, name=f"wb3_{img}")
    _emit_mac_pass(nc, ext3, wb3, 3, 261, 134, taps_sb, wbp, 3)

    def rhs3(q, c0, c1):
        return wb3[:, q, c0:c1]

    _emit_level_mm(nc, psp, detp, wsb, out, img, N=261, rhs=rhs3,
                   next_ext=None, det_names=("cH3", "cV3", "cD3"))


def _emit_level_mm(nc, psp, detp, wsb, out, img, N, rhs, next_ext, det_names):
    """height-pass matmuls + psum->sbuf quadrant copies + detail DMAs."""
    import concourse.mybir as mybir

    Np = nprime(N)
    arr, idx, per_t = WC[N]
    OFF = hi_off(Np)
    R = OFF + Np
    ot = (R + 127) // 128
    kN = N  # contraction length
    chunks = _free_chunks(Np)

    for t in range(ot):
        mt = min(128, R - t * 128)
        qs = per_t[t]
        ps_tiles = []
        for ci, (c0, c1) in enumerate(chunks):
            w = c1 - c0
            tag = "psA" if w > 256 else "psB"
            ps = psp.tile([128, w], mybir.dt.float32, tag=tag, bufs=4,
                          name=f"ps_{img}_{N}_{t}_{ci}")
            ps_tiles.append(ps)
            for ki, q in enumerate(qs):
                kq = min(128, kN - q * 128)
                r = rhs(q, c0, c1)
                if kq < 128:
                    r = r[0:kq]
                nc.tensor.matmul(
                    ps[0:mt, 0:w],
                    wsb[N][0:kq, idx[(t, q)], 0:mt],
                    r,
                    start=(ki == 0), stop=(ki == len(qs) - 1))

        # quadrant qd -> list of (ps_tile, dst_col0, width)
        quad_srcs = {0: [], 1: []}
        for ci, (c0, c1) in enumerate(chunks):
            qd = 0 if c0 < Np else 1
            quad_srcs[qd].append((ps_tiles[ci], c0 - qd * Np, c1 - c0))

        # lo rows: global [0:Np); hi rows: global [OFF:OFF+Np)
        lo_end = min(128, Np - t * 128) if t * 128 < Np else 0
        hp0 = max(0, OFF - t * 128)
        hp1 = max(0, min(128, OFF + Np - t * 128))
        # split hi ranges at legal partition starts (0/32/64)
        hi_ranges = []
        if hp0 < hp1:
            if hp0 == 0:
                hi_ranges = [(0, hp1)]
            else:
                assert hp0 == 32, hp0
                hi_ranges = [(32, min(64, hp1))]
                if hp1 > 64:
                    hi_ranges.append((64, hp1))

        if lo_end > 0:
            # quadrant 0 = aa -> next level ext (or cA3 staging tile)
            if next_ext is not None:
                for ps, d0, w in quad_srcs[0]:
                    nc.scalar.copy(out=next_ext[0:lo_end, t, 6 + d0:6 + d0 + w],
                                   in_=ps[0:lo_end, 0:w])
            else:
                _emit_det_copy_dma(nc, detp, out, img, "cA3", Np,
                                   quad_srcs[0], t, [(0, lo_end)], 0)
            # quadrant 1 = ad = cV
            _emit_det_copy_dma(nc, detp, out, img, det_names[1], Np,
                               quad_srcs[1], t, [(0, lo_end)], 0)
        if hi_ranges:
            # hi rows: da = cH (quadrant 0), dd = cD (quadrant 1)
            _emit_det_copy_dma(nc, detp, out, img, det_names[0], Np,
                               quad_srcs[0], t, hi_ranges, OFF)
            _emit_det_copy_dma(nc, detp, out, img, det_names[2], Np,
                               quad_srcs[1], t, hi_ranges, OFF)


def _emit_det_copy_dma(nc, detp, out, img, sec_name, Np, srcs, t, pranges,
                       row_off):
    """Copy psum chunks into a staging tile, then DMA rows to DRAM.

    h (row index within the detail) = 128*t + p - row_off for partition p.
    pranges: list of legal-start partition ranges covering this tile's rows."""
    import concourse.mybir as mybir
    sec_base, Wd = SECT[sec_name]
    assert Wd == Np
    p0, p1 = pranges[0][0], pranges[-1][1]
    h0 = 128 * t + p0 - row_off
    h1 = h0 + (p1 - p0)
    assert 0 <= h0 and h1 <= Np, (sec_name, t, pranges, h0, h1)

    is_cd1 = sec_name == "cD1"
    if is_cd1 and h0 >= CD1_FULL_ROWS + 1:
        return  # fully truncated
    dt = detp.tile([128, Np], mybir.dt.float32, tag=f"det{Np}", bufs=DET_BUFS,
                   name=f"det_{sec_name}_{img}_{t}_{p0}")
    for ps, d0, w in srcs:
        for (a, b) in pranges:
            nc.scalar.copy(out=dt[a:b, d0:d0 + w], in_=ps[a:b, 0:w])

    full_h1 = h1
    if is_cd1 and h1 > CD1_FULL_ROWS:
        full_h1 = CD1_FULL_ROWS
    if full_h1 > h0:
        npart = full_h1 - h0
        dst = out[img, sec_base + h0 * Wd: sec_base + full_h1 * Wd].rearrange(
            "(h w) -> h w", w=Wd)
        nc.sync.dma_start(out=dst, in_=dt[p0:p0 + npart, :])
    if is_cd1 and h0 <= CD1_FULL_ROWS < h1:
        pp = p0 + (CD1_FULL_ROWS - h0)
        dst = out[img, sec_base + CD1_FULL_ROWS * Wd:
                  sec_base + CD1_FULL_ROWS * Wd + CD1_PART_COLS]
        nc.sync.dma_start(out=dst.rearrange("(h w) -> h w", w=CD1_PART_COLS),
                          in_=dt[pp:pp + 1, 0:CD1_PART_COLS])


# ----------------------------------------------------------------- runner
def _get_built():
    global _BUILT
    if _BUILT is None:
        _BUILT = build_bass()
    return _BUILT


def kernel(x: np.ndarray) -> np.ndarray:
    from concourse import bass_utils

    x = np.ascontiguousarray(np.asarray(x), dtype=np.float32)
    assert x.shape == (B, C, H, W), x.shape
    nc = _get_built()

    imgs = x.reshape(B * C, H, W)
    in_maps = []
    for c in range(N_CORES):
        m = {"xin": imgs[c * IMGS_PER_CORE:(c + 1) * IMGS_PER_CORE]}
        for N, _, _, _ in LEVELS:
            m[f"w{N}"] = WC[N][0]
        m["taps"] = TAPS_ARR
        in_maps.append(m)

    res = bass_utils.run_bass_kernel_spmd(nc, in_maps,
                                          core_ids=list(range(N_CORES)))
    outs = [res.results[c]["out"] for c in range(N_CORES)]
    flat = np.concatenate(outs, axis=0)  # [48, 1048576]
    return flat.reshape(B, C, 64, 128, 128)
